# revision 2
# baseline (speedup 1.0000x reference)
"""Causal self-attention on 8 Trainium2 NeuronCores (Bass/Tile), v2.

Problem shape (hardcoded): x [2, 2048, 1024], W_attn [1024, 3072],
b_attn [3072], W_proj [1024, 1024], b_proj [1024], 16 heads, hd=64.

Sharding: tensor-parallel over (batch, head-group). Core k handles
batch k//4 and heads 4*(k%4) .. 4*(k%4)+3. Each core computes its 4
heads' attention and a partial output projection [2048, 1024] (bf16);
the host sums the four partials per batch.

v2 vs baseline:
- all matmuls bf16 (same PE cycles/row as f32r@512 but half DMA/SBUF)
- P@V in natural-y layout [128q, 65] (ap=65) instead of y^T (ap=512):
  halves the charged PE rows of the PV stage
- V computed in natural layout straight from the QKV projection (no PE
  transposes)
- causal mask folded into the PE: a tiny bf16 matmul accumulates a
  constant strict-upper-triangle -1e9 bias onto each diagonal S tile
- softmax sums arrive in a per-partition column via an appended
  ones-column of V; reciprocal + normalize-evict fused on DVE
- y^T for the projection via XBAR dma_start_transpose (SBUF->SBUF bf16)
- output DMA issued from gpsimd (SWDGE) to keep the SP DMA queue free
- S emission front-runs PV so the exp stream (Activation engine, the #2
  resource) starts the big late strips early; projection is fused per
  q-tile behind PV with a one-tile stagger so nothing lumps at the tail
"""

import sys

for _p in ("/opt/trn_rl_repo", "/root/.axon_site/_ro/trn_rl_repo"):
    if _p not in sys.path:
        sys.path.insert(0, _p)

import ml_dtypes
import numpy as np

import concourse.bass as bass  # noqa: F401
import concourse.mybir as mybir
import concourse.tile as tile
from concourse import bacc
from concourse.bass_utils import run_bass_kernel_spmd

F32 = mybir.dt.float32
BF16 = mybir.dt.bfloat16
BF16NP = ml_dtypes.bfloat16

B = 2
T = 2048
C = 1024
H = 16
HD = 64
NCORES = 8
HEADS_PER_CORE = 4
PAIRS = 2
NKT = T // 128       # 16 k-tiles (= t-tiles)
NST = T // 512       # 4 q-strips
CKT = C // 128       # 8 contraction tiles over C

_CACHE = {}


def _build():
    nc = bacc.Bacc(None, target_bir_lowering=False)

    xt_d = nc.dram_tensor("xt", [C, T], BF16, kind="ExternalInput")
    wq_d = nc.dram_tensor("wq", [128, CKT, 256], BF16, kind="ExternalInput")
    wk_d = nc.dram_tensor("wk", [128, CKT, 256], BF16, kind="ExternalInput")
    wv_d = nc.dram_tensor("wv", [128, CKT, 256], BF16, kind="ExternalInput")
    wp_d = nc.dram_tensor("wp", [128, PAIRS, C], BF16, kind="ExternalInput")
    id_d = nc.dram_tensor("ident", [128, 128], BF16, kind="ExternalInput")
    mask_d = nc.dram_tensor("masktri", [128, 128], BF16, kind="ExternalInput")
    out_d = nc.dram_tensor("out", [T, C], BF16, kind="ExternalOutput")

    with tile.TileContext(nc) as tc, (
        tc.tile_pool(name="const", bufs=1)
    ) as const, (
        tc.tile_pool(name="weights", bufs=1)
    ) as wpool, (
        tc.tile_pool(name="acts", bufs=1)
    ) as apool, (
        tc.tile_pool(name="xstream", bufs=3)
    ) as xpool, (
        tc.tile_pool(name="ptiles", bufs=56)
    ) as ppool, (
        tc.tile_pool(name="evict", bufs=3)
    ) as epool, (
        tc.tile_pool(name="st_ps", bufs=3, space="PSUM")
    ) as st_ps, (
        tc.tile_pool(name="mm_ps", bufs=2, space="PSUM")
    ) as mm_ps:
        ident = const.tile([128, 128], BF16)
        masktri = const.tile([128, 128], BF16)

        wq = wpool.tile([128, CKT, 256], BF16)
        wk = wpool.tile([128, CKT, 256], BF16)
        wv = wpool.tile([128, CKT, 256], BF16)
        wp = wpool.tile([128, PAIRS, C], BF16)

        # resident activations (all bf16)
        qt = apool.tile([128, PAIRS, T], BF16)       # q^T pairs
        kt = apool.tile([128, PAIRS, T], BF16)       # k^T pairs
        v_nat = apool.tile([128, NKT, HEADS_PER_CORE, HD + 1], BF16)
        y_sb = apool.tile([128, NKT, HEADS_PER_CORE, HD], BF16)  # normalized y
        yT = apool.tile([128, PAIRS, T], BF16)       # y^T pairs (via XBAR)
        recip = apool.tile([128, NKT, HEADS_PER_CORE], F32)

        # ones column of v_nat (softmax-sum trick), written once
        nc.gpsimd.memset(v_nat[:, :, :, HD:HD + 1], 1.0)

        xs = [None] * NST
        # P[s][h][g] : bf16 exp(S^T) tiles, 2 k-tiles per group
        P = [[[None] * (NKT // 2) for _ in range(HEADS_PER_CORE)] for _ in range(NST)]

        xt_r = xt_d[:].rearrange("(c p) t -> p c t", p=128)

        def load_x(s, nsplit=1):
            xc = xpool.tile([128, CKT, 512], BF16, name=f"x_{s}", tag="x")
            step = CKT // nsplit
            for i in range(nsplit):
                nc.sync.dma_start(
                    xc[:, i * step:(i + 1) * step, :],
                    xt_r[:, i * step:(i + 1) * step, s * 512:(s + 1) * 512],
                )
            xs[s] = xc

        def qk_item(s, p, wi):
            w_t, dest = ((wq, qt), (wk, kt))[wi]
            ps = mm_ps.tile([128, 512], F32, name=f"qkps_{s}_{p}_{wi}", tag="mm")
            for kc in range(CKT):
                nc.tensor.matmul(
                    ps[:],
                    w_t[:, kc, p * 128:(p + 1) * 128],
                    xs[s][:, kc, :],
                    start=(kc == 0),
                    stop=(kc == CKT - 1),
                )
            nc.vector.tensor_copy(dest[:, p, s * 512:(s + 1) * 512], ps[:])

        def v_item(s, t):
            ps = mm_ps.tile([128, 256], F32, name=f"vps_{t}", tag="mm")
            for kc in range(CKT):
                nc.tensor.matmul(
                    ps[:],
                    xs[s][:, kc, (t - 4 * s) * 128:(t - 4 * s) * 128 + 128],
                    wv[:, kc, :],
                    start=(kc == 0),
                    stop=(kc == CKT - 1),
                )
            nc.vector.tensor_copy(
                v_nat[:, t, :, 0:HD], ps[:].rearrange("p (h d) -> p h d", h=4)
            )

        def s_unit(s, g, h):
            """One S^T group (+ causal bias) + exp: strip s, group g, head h."""
            p, hh = divmod(h, 2)
            st = st_ps.tile([128, 1024], F32, name=f"st_{s}_{g}_{h}", tag="st")
            for jj in range(2):
                j = 2 * g + jj
                c0 = max(0, 128 * (j - 4 * s))
                nc.tensor.matmul(
                    st[:, jj * 512 + c0:(jj + 1) * 512],
                    kt[hh * HD:(hh + 1) * HD, p, j * 128:(j + 1) * 128],
                    qt[hh * HD:(hh + 1) * HD, p, s * 512 + c0:(s + 1) * 512],
                    start=True,
                    stop=(j < 4 * s),
                )
                if j >= 4 * s:  # diagonal tile: accumulate -1e9 triangle
                    nc.tensor.matmul(
                        st[:, jj * 512 + c0:jj * 512 + c0 + 128],
                        masktri[:],
                        ident[:],
                        start=False,
                        stop=True,
                    )
            pt = ppool.tile([128, 1024], BF16, name=f"P_{s}_{g}_{h}", tag="P")
            # exp only the causally-live columns: [c0e, 512) of the even
            # tile and [512+c0o, 1024) of the odd tile (dead columns are
            # never read by PV)
            c0e = max(0, 128 * (2 * g - 4 * s))
            c0o = max(0, 128 * (2 * g + 1 - 4 * s))
            if c0o >= 256:
                nc.scalar.activation(
                    pt[:, c0e:512], st[:, c0e:512], mybir.ActivationFunctionType.Exp
                )
                nc.scalar.activation(
                    pt[:, 512 + c0o:], st[:, 512 + c0o:], mybir.ActivationFunctionType.Exp
                )
            else:
                nc.scalar.activation(
                    pt[:, c0e:], st[:, c0e:], mybir.ActivationFunctionType.Exp
                )
            P[s][h][g] = pt

        def s_list(s):
            """S-units of strip s. Pair-0 heads lead (they only need the
            pair-0 q/k projections), group-major within each half; the
            last strip goes fully group-major so the epilogue's pv tiles
            unblock as early as possible in the exp stream."""
            n_g = 2 * s + 2
            return [(s, g, h) for hs in ((0, 1), (2, 3)) for g in range(n_g) for h in hs]

        def tr_item(qi, on_act=False):
            """Epilogue y^T via PE transpose + engine evict (act is idle at
            the tail, and this skips the ~2.4us DMA-transpose latency)."""
            tp = mm_ps.tile([128, 256], BF16, name=f"trp_{qi}", tag="mm")
            for p in range(PAIRS):
                nc.tensor.transpose(
                    tp[:, p * 128:(p + 1) * 128],
                    y_sb[:, qi, 2 * p:2 * p + 2, :],
                    ident[:],
                )
            dst = yT[:, :, qi * 128:(qi + 1) * 128]
            src = tp[:].rearrange("p (a b) -> p a b", a=2)
            if on_act:
                nc.scalar.copy(dst, src)
            else:
                nc.vector.tensor_copy(dst, src)

        def pv_tile(s, i, on_act=False, do_tr=True):
            """PV + normalize + y^T transpose for q-tile 4s+i."""
            qi = 4 * s + i
            n_k = qi + 1
            yt = mm_ps.tile([128, HEADS_PER_CORE, HD + 1], F32, name=f"yt_{qi}", tag="mm")
            for h in range(HEADS_PER_CORE):
                for j in range(n_k):
                    nc.tensor.matmul(
                        yt[:, h, :],
                        P[s][h][j // 2][:, (j % 2) * 512 + i * 128:(j % 2) * 512 + i * 128 + 128],
                        v_nat[:, j, h, :],
                        start=(j == 0),
                        stop=(j == n_k - 1),
                    )
            nc.vector.reciprocal_approx_fast(recip[:, qi, :], yt[:, :, HD])
            for p in range(PAIRS):
                for h in (2 * p, 2 * p + 1):
                    if on_act:  # epilogue: act engine is idle, DVE is not
                        nc.scalar.mul(
                            y_sb[:, qi, h, :], yt[:, h, 0:HD], recip[:, qi, h:h + 1]
                        )
                    else:
                        nc.vector.tensor_scalar_mul(
                            y_sb[:, qi, h, :], yt[:, h, 0:HD], recip[:, qi, h:h + 1]
                        )
                if do_tr:
                    nc.sync.dma_start_transpose(
                        yT[:, p, qi * 128:(qi + 1) * 128],
                        y_sb[:, qi, 2 * p:2 * p + 2, :],
                    )

        def proj_tile(t, split_dma=False, on_act=False):
            ot = epool.tile([128, 1024], BF16, name=f"ot_{t}", tag="ot")
            for n in range(2):
                op = mm_ps.tile([128, 512], F32, name=f"op_{t}_{n}", tag="mm")
                for f in range(PAIRS):
                    nc.tensor.matmul(
                        op[:],
                        yT[:, f, t * 128:(t + 1) * 128],
                        wp[:, f, n * 512:(n + 1) * 512],
                        start=(f == 0),
                        stop=(f == PAIRS - 1),
                    )
                if on_act:
                    nc.scalar.copy(ot[:, n * 512:(n + 1) * 512], op[:])
                else:
                    nc.vector.tensor_copy(ot[:, n * 512:(n + 1) * 512], op[:])
                if split_dma:  # tail: overlap the out DMA with the 2nd evict
                    nc.sync.dma_start(
                        out_d[t * 128:(t + 1) * 128, n * 512:(n + 1) * 512],
                        ot[:, n * 512:(n + 1) * 512],
                    )
            if not split_dma:
                nc.gpsimd.dma_start(out_d[t * 128:(t + 1) * 128, :], ot[:])

        def weave(s_units, fillers, forced=2):
            """Interleave act-feeding S-units with PE filler work so
            neither engine starves: fillers spread evenly through the
            S stream (first `forced` fillers lead)."""
            for f in fillers[:forced]:
                f()
            rest = fillers[forced:]
            nS, nF = len(s_units), len(rest)
            si = 0
            for j in range(nF):
                target = ((j + 1) * nS) // (nF + 1)
                while si < target:
                    s_unit(*s_units[si])
                    si += 1
                rest[j]()
            while si < nS:
                s_unit(*s_units[si])
                si += 1

        # ---- DMA order: first-needed first (HWDGE is serial); wq and x
        # interleaved in contraction order so the first matmuls start early
        x0 = xpool.tile([128, CKT, 512], BF16, name="x_0", tag="x")
        xs[0] = x0
        nc.sync.dma_start(wq[:, 0:2, :], wq_d[:, 0:2, :])
        nc.sync.dma_start(x0[:, 0:2, :], xt_r[:, 0:2, 0:512])
        nc.sync.dma_start(wq[:, 2:8, :], wq_d[:, 2:8, :])
        for i in range(1, 4):
            nc.sync.dma_start(
                x0[:, 2 * i:2 * i + 2, :], xt_r[:, 2 * i:2 * i + 2, 0:512]
            )
        nc.sync.dma_start(wk[:], wk_d[:])
        nc.sync.dma_start(ident[:], id_d[:])
        nc.sync.dma_start(masktri[:], mask_d[:])
        nc.sync.dma_start(wv[:], wv_d[:])
        nc.sync.dma_start(wp[:], wp_d[:])
        load_x(1, nsplit=2)

        # ---- schedule: qk+S backbones run as early as possible so the
        # Activation engine's exp stream (83us, the #2 resource) is
        # front-loaded; v/pv/proj items are woven between S-units at the
        # rate that keeps PE fed while act drains, each placed at least
        # one S-run after its dependencies (PE is in-order: a stalled
        # item blocks everything behind it) ----
        def F(fn, *a, **k):
            return lambda: fn(*a, **k)

        # per-strip filler chunks, dependency-ordered; the strip's own
        # pair-1 q/k projections lead each chunk so the pair-0 S-units
        # (first half of s_list) can start right after the pair-0 items
        def qk1(s):
            return [F(qk_item, s, 1, 0), F(qk_item, s, 1, 1)]

        chunks = [
            qk1(0) + [F(v_item, 0, t) for t in range(4)],
            qk1(1) + [F(v_item, 1, t) for t in range(4, 8)]
            + [F(pv_tile, 0, i) for i in range(4)],
            qk1(2) + [F(v_item, 2, t) for t in range(8, 12)]
            + [F(proj_tile, t) for t in range(4)]
            + [F(pv_tile, 1, i) for i in range(4)],
            qk1(3) + [F(v_item, 3, t) for t in range(12, 16)]
            + [F(proj_tile, t) for t in range(4, 8)]
            + [F(pv_tile, 2, i) for i in range(4)]
            + [F(proj_tile, t) for t in range(8, 12)],
        ]

        def qk_pair_interleaved(s, p):
            """q and k matmuls interleaved per contraction tile: at startup
            each arriving x chunk feeds both accumulations immediately."""
            psq = mm_ps.tile([128, 512], F32, name=f"qkps_{s}_{p}_0", tag="mm")
            psk = mm_ps.tile([128, 512], F32, name=f"qkps_{s}_{p}_1", tag="mm")
            for kc in range(CKT):
                for w_t, ps in ((wq, psq), (wk, psk)):
                    nc.tensor.matmul(
                        ps[:],
                        w_t[:, kc, p * 128:(p + 1) * 128],
                        xs[s][:, kc, :],
                        start=(kc == 0),
                        stop=(kc == CKT - 1),
                    )
            nc.vector.tensor_copy(qt[:, p, s * 512:(s + 1) * 512], psq[:])
            nc.vector.tensor_copy(kt[:, p, s * 512:(s + 1) * 512], psk[:])

        for s in range(NST):
            if s + 1 < NST and s > 0:
                load_x(s + 1)
            if s == 0:
                qk_pair_interleaved(0, 0)
            else:
                qk_item(s, 0, 0)
                qk_item(s, 0, 1)
            weave(s_list(s), chunks[s], forced=0)
        # epilogue: last strip's pv/proj; normalize/evictions go to the
        # now-idle act engine, final out-DMAs split to overlap evictions
        for i in range(4):
            pv_tile(3, i, on_act=(i >= 2))
        for i in range(4):
            proj_tile(12 + i, split_dma=(i >= 2), on_act=(i >= 2))

    nc.compile()
    return nc


def _prep_inputs(x, W_attn, b_attn, W_proj):
    """Per-core input maps. Core k: batch k//4, head-group k%4."""
    assert np.allclose(b_attn, 0.0), "nonzero b_attn not supported by this kernel"
    scale = 1.0 / np.sqrt(np.float32(HD))

    ident = np.eye(128, dtype=np.float32).astype(BF16NP)
    idx = np.arange(128)
    # masktri[q, k] = -1e9 where k > q (strict upper triangle)
    masktri = np.where(idx[None, :] > idx[:, None], np.float32(-1e9), np.float32(0.0))
    masktri = masktri.astype(BF16NP)

    in_maps = []
    for core in range(NCORES):
        b = core // 4
        g = core % 4
        heads = [4 * g + i for i in range(HEADS_PER_CORE)]
        cols = np.concatenate([np.arange(h * HD, (h + 1) * HD) for h in heads])

        xt = np.ascontiguousarray(x[b].T).astype(BF16NP)

        def lhsT256(w):
            # [C, 256] -> [128, CKT, 256] with [p, ct, f] = w[ct*128+p, f]
            return np.ascontiguousarray(w.reshape(CKT, 128, 256).transpose(1, 0, 2))

        wq = lhsT256(W_attn[:, cols] * scale).astype(BF16NP)
        wk = lhsT256(W_attn[:, C + cols]).astype(BF16NP)
        # natural v weights (rhs layout): same [128, CKT, 256] but these are
        # used as rhs with partition = contraction, so identical transform
        wv = lhsT256(W_attn[:, 2 * C + cols]).astype(BF16NP)
        wp_rows = W_proj[heads[0] * HD:(heads[-1] + 1) * HD, :]
        wp = np.ascontiguousarray(
            wp_rows.reshape(2, 128, C).transpose(1, 0, 2)
        ).astype(BF16NP)

        in_maps.append(
            {
                "xt": xt,
                "wq": np.ascontiguousarray(wq),
                "wk": np.ascontiguousarray(wk),
                "wv": np.ascontiguousarray(wv),
                "wp": np.ascontiguousarray(wp),
                "ident": ident,
                "masktri": masktri,
            }
        )
    return in_maps


def kernel(x, W_attn, b_attn, W_proj, b_proj, _want_results=False, _spmd_kwargs=None):
    x = np.asarray(x, dtype=np.float32)
    W_attn = np.asarray(W_attn, dtype=np.float32)
    b_attn = np.asarray(b_attn, dtype=np.float32)
    W_proj = np.asarray(W_proj, dtype=np.float32)
    b_proj = np.asarray(b_proj, dtype=np.float32)

    if "nc" not in _CACHE:
        _CACHE["nc"] = _build()
    nc = _CACHE["nc"]

    in_maps = _prep_inputs(x, W_attn, b_attn, W_proj)
    kw = dict(_spmd_kwargs or {})
    res = run_bass_kernel_spmd(nc, in_maps, list(range(NCORES)), **kw)

    out = np.zeros((B, T, C), dtype=np.float32)
    for core in range(NCORES):
        out[core // 4] += res.results[core]["out"].astype(np.float32)
    out += b_proj[None, None, :]
    if _want_results:
        return out, res
    return out


# revision 3
# speedup vs baseline: 1.0002x; 1.0002x over previous
"""Causal self-attention on 8 Trainium2 NeuronCores (Bass/Tile), v2.

Problem shape (hardcoded): x [2, 2048, 1024], W_attn [1024, 3072],
b_attn [3072], W_proj [1024, 1024], b_proj [1024], 16 heads, hd=64.

Sharding: tensor-parallel over (batch, head-group). Core k handles
batch k//4 and heads 4*(k%4) .. 4*(k%4)+3. Each core computes its 4
heads' attention and a partial output projection [2048, 1024] (bf16);
the host sums the four partials per batch.

v2 vs baseline:
- all matmuls bf16 (same PE cycles/row as f32r@512 but half DMA/SBUF)
- P@V in natural-y layout [128q, 65] (ap=65) instead of y^T (ap=512):
  halves the charged PE rows of the PV stage
- V computed in natural layout straight from the QKV projection (no PE
  transposes)
- causal mask folded into the PE: a tiny bf16 matmul accumulates a
  constant strict-upper-triangle -1e9 bias onto each diagonal S tile
- softmax sums arrive in a per-partition column via an appended
  ones-column of V; reciprocal + normalize-evict fused on DVE
- y^T for the projection via XBAR dma_start_transpose (SBUF->SBUF bf16)
- output DMA issued from gpsimd (SWDGE) to keep the SP DMA queue free
- S emission front-runs PV so the exp stream (Activation engine, the #2
  resource) starts the big late strips early; projection is fused per
  q-tile behind PV with a one-tile stagger so nothing lumps at the tail
"""

import sys

for _p in ("/opt/trn_rl_repo", "/root/.axon_site/_ro/trn_rl_repo"):
    if _p not in sys.path:
        sys.path.insert(0, _p)

import ml_dtypes
import numpy as np

import concourse.bass as bass  # noqa: F401
import concourse.mybir as mybir
import concourse.tile as tile
from concourse import bacc
from concourse.bass_utils import run_bass_kernel_spmd

F32 = mybir.dt.float32
BF16 = mybir.dt.bfloat16
BF16NP = ml_dtypes.bfloat16

B = 2
T = 2048
C = 1024
H = 16
HD = 64
NCORES = 8
HEADS_PER_CORE = 4
PAIRS = 2
NKT = T // 128       # 16 k-tiles (= t-tiles)
NST = T // 512       # 4 q-strips
CKT = C // 128       # 8 contraction tiles over C

_CACHE = {}


def _build():
    nc = bacc.Bacc(None, target_bir_lowering=False)

    xt_d = nc.dram_tensor("xt", [C, T], BF16, kind="ExternalInput")
    wq_d = nc.dram_tensor("wq", [128, CKT, 256], BF16, kind="ExternalInput")
    wk_d = nc.dram_tensor("wk", [128, CKT, 256], BF16, kind="ExternalInput")
    wv_d = nc.dram_tensor("wv", [128, CKT, 256], BF16, kind="ExternalInput")
    wp_d = nc.dram_tensor("wp", [128, PAIRS, C], BF16, kind="ExternalInput")
    id_d = nc.dram_tensor("ident", [128, 128], BF16, kind="ExternalInput")
    mask_d = nc.dram_tensor("masktri", [128, 128], BF16, kind="ExternalInput")
    out_d = nc.dram_tensor("out", [T, C], BF16, kind="ExternalOutput")

    with tile.TileContext(nc) as tc, (
        tc.tile_pool(name="const", bufs=1)
    ) as const, (
        tc.tile_pool(name="weights", bufs=1)
    ) as wpool, (
        tc.tile_pool(name="acts", bufs=1)
    ) as apool, (
        tc.tile_pool(name="xstream", bufs=3)
    ) as xpool, (
        tc.tile_pool(name="ptiles", bufs=56)
    ) as ppool, (
        tc.tile_pool(name="evict", bufs=6)
    ) as epool, (
        tc.tile_pool(name="st_ps", bufs=3, space="PSUM")
    ) as st_ps, (
        tc.tile_pool(name="mm_ps", bufs=2, space="PSUM")
    ) as mm_ps:
        ident = const.tile([128, 128], BF16)
        masktri = const.tile([128, 128], BF16)

        wq = wpool.tile([128, CKT, 256], BF16)
        wk = wpool.tile([128, CKT, 256], BF16)
        wv = wpool.tile([128, CKT, 256], BF16)
        wp = wpool.tile([128, PAIRS, C], BF16)

        # resident activations (all bf16)
        qt = apool.tile([128, PAIRS, T], BF16)       # q^T pairs
        kt = apool.tile([128, PAIRS, T], BF16)       # k^T pairs
        v_nat = apool.tile([128, NKT, HEADS_PER_CORE, HD + 1], BF16)
        y_sb = apool.tile([128, NKT, HEADS_PER_CORE, HD], BF16)  # normalized y
        yT = apool.tile([128, PAIRS, T], BF16)       # y^T pairs (via XBAR)
        recip = apool.tile([128, NKT, HEADS_PER_CORE], F32)

        # ones column of v_nat (softmax-sum trick), written once
        nc.gpsimd.memset(v_nat[:, :, :, HD:HD + 1], 1.0)

        xs = [None] * NST
        # P[s][h][g] : bf16 exp(S^T) tiles, 2 k-tiles per group
        P = [[[None] * (NKT // 2) for _ in range(HEADS_PER_CORE)] for _ in range(NST)]

        xt_r = xt_d[:].rearrange("(c p) t -> p c t", p=128)

        def load_x(s, nsplit=1):
            xc = xpool.tile([128, CKT, 512], BF16, name=f"x_{s}", tag="x")
            step = CKT // nsplit
            for i in range(nsplit):
                nc.sync.dma_start(
                    xc[:, i * step:(i + 1) * step, :],
                    xt_r[:, i * step:(i + 1) * step, s * 512:(s + 1) * 512],
                )
            xs[s] = xc

        def qk_item(s, p, wi):
            w_t, dest = ((wq, qt), (wk, kt))[wi]
            ps = mm_ps.tile([128, 512], F32, name=f"qkps_{s}_{p}_{wi}", tag="mm")
            for kc in range(CKT):
                nc.tensor.matmul(
                    ps[:],
                    w_t[:, kc, p * 128:(p + 1) * 128],
                    xs[s][:, kc, :],
                    start=(kc == 0),
                    stop=(kc == CKT - 1),
                )
            nc.vector.tensor_copy(dest[:, p, s * 512:(s + 1) * 512], ps[:])

        def v_item(s, t):
            ps = mm_ps.tile([128, 256], F32, name=f"vps_{t}", tag="mm")
            for kc in range(CKT):
                nc.tensor.matmul(
                    ps[:],
                    xs[s][:, kc, (t - 4 * s) * 128:(t - 4 * s) * 128 + 128],
                    wv[:, kc, :],
                    start=(kc == 0),
                    stop=(kc == CKT - 1),
                )
            nc.vector.tensor_copy(
                v_nat[:, t, :, 0:HD], ps[:].rearrange("p (h d) -> p h d", h=4)
            )

        def s_unit(s, g, h):
            """One S^T group (+ causal bias) + exp: strip s, group g, head h."""
            p, hh = divmod(h, 2)
            st = st_ps.tile([128, 1024], F32, name=f"st_{s}_{g}_{h}", tag="st")
            for jj in range(2):
                j = 2 * g + jj
                c0 = max(0, 128 * (j - 4 * s))
                nc.tensor.matmul(
                    st[:, jj * 512 + c0:(jj + 1) * 512],
                    kt[hh * HD:(hh + 1) * HD, p, j * 128:(j + 1) * 128],
                    qt[hh * HD:(hh + 1) * HD, p, s * 512 + c0:(s + 1) * 512],
                    start=True,
                    stop=(j < 4 * s),
                )
                if j >= 4 * s:  # diagonal tile: accumulate -1e9 triangle
                    nc.tensor.matmul(
                        st[:, jj * 512 + c0:jj * 512 + c0 + 128],
                        masktri[:],
                        ident[:],
                        start=False,
                        stop=True,
                    )
            pt = ppool.tile([128, 1024], BF16, name=f"P_{s}_{g}_{h}", tag="P")
            # exp only the causally-live columns: [c0e, 512) of the even
            # tile and [512+c0o, 1024) of the odd tile (dead columns are
            # never read by PV)
            c0e = max(0, 128 * (2 * g - 4 * s))
            c0o = max(0, 128 * (2 * g + 1 - 4 * s))
            if c0o >= 256:
                nc.scalar.activation(
                    pt[:, c0e:512], st[:, c0e:512], mybir.ActivationFunctionType.Exp
                )
                nc.scalar.activation(
                    pt[:, 512 + c0o:], st[:, 512 + c0o:], mybir.ActivationFunctionType.Exp
                )
            else:
                nc.scalar.activation(
                    pt[:, c0e:], st[:, c0e:], mybir.ActivationFunctionType.Exp
                )
            P[s][h][g] = pt

        def s_list(s):
            """S-units of strip s. Pair-0 heads lead (they only need the
            pair-0 q/k projections), group-major within each half; the
            last strip goes fully group-major so the epilogue's pv tiles
            unblock as early as possible in the exp stream."""
            n_g = 2 * s + 2
            return [(s, g, h) for hs in ((0, 1), (2, 3)) for g in range(n_g) for h in hs]

        def tr_item(qi, on_act=False):
            """Epilogue y^T via PE transpose + engine evict (act is idle at
            the tail, and this skips the ~2.4us DMA-transpose latency)."""
            tp = mm_ps.tile([128, 256], BF16, name=f"trp_{qi}", tag="mm")
            for p in range(PAIRS):
                nc.tensor.transpose(
                    tp[:, p * 128:(p + 1) * 128],
                    y_sb[:, qi, 2 * p:2 * p + 2, :],
                    ident[:],
                )
            dst = yT[:, :, qi * 128:(qi + 1) * 128]
            src = tp[:].rearrange("p (a b) -> p a b", a=2)
            if on_act:
                nc.scalar.copy(dst, src)
            else:
                nc.vector.tensor_copy(dst, src)

        def pv_tile(s, i, on_act=False, do_tr=True):
            """PV + normalize + y^T transpose for q-tile 4s+i."""
            qi = 4 * s + i
            n_k = qi + 1
            yt = mm_ps.tile([128, HEADS_PER_CORE, HD + 1], F32, name=f"yt_{qi}", tag="mm")
            for h in range(HEADS_PER_CORE):
                for j in range(n_k):
                    nc.tensor.matmul(
                        yt[:, h, :],
                        P[s][h][j // 2][:, (j % 2) * 512 + i * 128:(j % 2) * 512 + i * 128 + 128],
                        v_nat[:, j, h, :],
                        start=(j == 0),
                        stop=(j == n_k - 1),
                    )
            nc.vector.reciprocal_approx_fast(recip[:, qi, :], yt[:, :, HD])
            for p in range(PAIRS):
                for h in (2 * p, 2 * p + 1):
                    if on_act:  # epilogue: act engine is idle, DVE is not
                        nc.scalar.mul(
                            y_sb[:, qi, h, :], yt[:, h, 0:HD], recip[:, qi, h:h + 1]
                        )
                    else:
                        nc.vector.tensor_scalar_mul(
                            y_sb[:, qi, h, :], yt[:, h, 0:HD], recip[:, qi, h:h + 1]
                        )
                if do_tr:
                    nc.sync.dma_start_transpose(
                        yT[:, p, qi * 128:(qi + 1) * 128],
                        y_sb[:, qi, 2 * p:2 * p + 2, :],
                    )

        def proj_tile(t, split_dma=False, on_act=False):
            ot = epool.tile([128, 1024], BF16, name=f"ot_{t}", tag="ot")
            for n in range(2):
                op = mm_ps.tile([128, 512], F32, name=f"op_{t}_{n}", tag="mm")
                for f in range(PAIRS):
                    nc.tensor.matmul(
                        op[:],
                        yT[:, f, t * 128:(t + 1) * 128],
                        wp[:, f, n * 512:(n + 1) * 512],
                        start=(f == 0),
                        stop=(f == PAIRS - 1),
                    )
                if on_act:
                    nc.scalar.copy(ot[:, n * 512:(n + 1) * 512], op[:])
                else:
                    nc.vector.tensor_copy(ot[:, n * 512:(n + 1) * 512], op[:])
                if split_dma:  # tail: overlap the out DMA with the 2nd evict
                    nc.sync.dma_start(
                        out_d[t * 128:(t + 1) * 128, n * 512:(n + 1) * 512],
                        ot[:, n * 512:(n + 1) * 512],
                    )
            if not split_dma:
                nc.gpsimd.dma_start(out_d[t * 128:(t + 1) * 128, :], ot[:])

        def weave(s_units, fillers, forced=2):
            """Interleave act-feeding S-units with PE filler work so
            neither engine starves: fillers spread evenly through the
            S stream (first `forced` fillers lead)."""
            for f in fillers[:forced]:
                f()
            rest = fillers[forced:]
            nS, nF = len(s_units), len(rest)
            si = 0
            for j in range(nF):
                target = ((j + 1) * nS) // (nF + 1)
                while si < target:
                    s_unit(*s_units[si])
                    si += 1
                rest[j]()
            while si < nS:
                s_unit(*s_units[si])
                si += 1

        # ---- DMA order: first-needed first (HWDGE is serial); wq and x
        # interleaved in contraction order so the first matmuls start early
        x0 = xpool.tile([128, CKT, 512], BF16, name="x_0", tag="x")
        xs[0] = x0
        nc.sync.dma_start(wq[:, 0:2, :], wq_d[:, 0:2, :])
        nc.sync.dma_start(x0[:, 0:2, :], xt_r[:, 0:2, 0:512])
        nc.sync.dma_start(wq[:, 2:8, :], wq_d[:, 2:8, :])
        for i in range(1, 4):
            nc.sync.dma_start(
                x0[:, 2 * i:2 * i + 2, :], xt_r[:, 2 * i:2 * i + 2, 0:512]
            )
        nc.sync.dma_start(wk[:], wk_d[:])
        nc.sync.dma_start(ident[:], id_d[:])
        nc.sync.dma_start(masktri[:], mask_d[:])
        nc.sync.dma_start(wv[:], wv_d[:])
        nc.sync.dma_start(wp[:], wp_d[:])
        load_x(1, nsplit=2)

        # ---- schedule: qk+S backbones run as early as possible so the
        # Activation engine's exp stream (83us, the #2 resource) is
        # front-loaded; v/pv/proj items are woven between S-units at the
        # rate that keeps PE fed while act drains, each placed at least
        # one S-run after its dependencies (PE is in-order: a stalled
        # item blocks everything behind it) ----
        def F(fn, *a, **k):
            return lambda: fn(*a, **k)

        # per-strip filler chunks, dependency-ordered; the strip's own
        # pair-1 q/k projections lead each chunk so the pair-0 S-units
        # (first half of s_list) can start right after the pair-0 items
        def qk1(s):
            return [F(qk_item, s, 1, 0), F(qk_item, s, 1, 1)]

        chunks = [
            qk1(0) + [F(v_item, 0, t) for t in range(4)],
            qk1(1) + [F(v_item, 1, t) for t in range(4, 8)]
            + [F(pv_tile, 0, i) for i in range(4)],
            qk1(2) + [F(v_item, 2, t) for t in range(8, 12)]
            + [F(proj_tile, t) for t in range(4)]
            + [F(pv_tile, 1, i) for i in range(4)],
            qk1(3) + [F(v_item, 3, t) for t in range(12, 16)]
            + [F(proj_tile, t) for t in range(4, 8)]
            + [F(pv_tile, 2, i) for i in range(4)]
            + [F(proj_tile, t) for t in range(8, 12)],
        ]

        def qk_pair_interleaved(s, p):
            """q and k matmuls interleaved per contraction tile: at startup
            each arriving x chunk feeds both accumulations immediately."""
            psq = mm_ps.tile([128, 512], F32, name=f"qkps_{s}_{p}_0", tag="mm")
            psk = mm_ps.tile([128, 512], F32, name=f"qkps_{s}_{p}_1", tag="mm")
            for kc in range(CKT):
                for w_t, ps in ((wq, psq), (wk, psk)):
                    nc.tensor.matmul(
                        ps[:],
                        w_t[:, kc, p * 128:(p + 1) * 128],
                        xs[s][:, kc, :],
                        start=(kc == 0),
                        stop=(kc == CKT - 1),
                    )
            nc.vector.tensor_copy(qt[:, p, s * 512:(s + 1) * 512], psq[:])
            nc.vector.tensor_copy(kt[:, p, s * 512:(s + 1) * 512], psk[:])

        for s in range(NST):
            if s + 1 < NST and s > 0:
                load_x(s + 1)
            if s == 0:
                qk_pair_interleaved(0, 0)
            else:
                qk_item(s, 0, 0)
                qk_item(s, 0, 1)
            weave(s_list(s), chunks[s], forced=0)
        # epilogue: last strip's pv/proj; normalize/evictions go to the
        # now-idle act engine, final out-DMAs split to overlap evictions
        for i in range(4):
            pv_tile(3, i, on_act=(i >= 2))
        for i in range(4):
            proj_tile(12 + i, split_dma=(i >= 1), on_act=(i >= 2))

    nc.compile()
    return nc


def _prep_inputs(x, W_attn, b_attn, W_proj):
    """Per-core input maps. Core k: batch k//4, head-group k%4."""
    assert np.allclose(b_attn, 0.0), "nonzero b_attn not supported by this kernel"
    scale = 1.0 / np.sqrt(np.float32(HD))

    ident = np.eye(128, dtype=np.float32).astype(BF16NP)
    idx = np.arange(128)
    # masktri[q, k] = -1e9 where k > q (strict upper triangle)
    masktri = np.where(idx[None, :] > idx[:, None], np.float32(-1e9), np.float32(0.0))
    masktri = masktri.astype(BF16NP)

    in_maps = []
    for core in range(NCORES):
        b = core // 4
        g = core % 4
        heads = [4 * g + i for i in range(HEADS_PER_CORE)]
        cols = np.concatenate([np.arange(h * HD, (h + 1) * HD) for h in heads])

        xt = np.ascontiguousarray(x[b].T).astype(BF16NP)

        def lhsT256(w):
            # [C, 256] -> [128, CKT, 256] with [p, ct, f] = w[ct*128+p, f]
            return np.ascontiguousarray(w.reshape(CKT, 128, 256).transpose(1, 0, 2))

        wq = lhsT256(W_attn[:, cols] * scale).astype(BF16NP)
        wk = lhsT256(W_attn[:, C + cols]).astype(BF16NP)
        # natural v weights (rhs layout): same [128, CKT, 256] but these are
        # used as rhs with partition = contraction, so identical transform
        wv = lhsT256(W_attn[:, 2 * C + cols]).astype(BF16NP)
        wp_rows = W_proj[heads[0] * HD:(heads[-1] + 1) * HD, :]
        wp = np.ascontiguousarray(
            wp_rows.reshape(2, 128, C).transpose(1, 0, 2)
        ).astype(BF16NP)

        in_maps.append(
            {
                "xt": xt,
                "wq": np.ascontiguousarray(wq),
                "wk": np.ascontiguousarray(wk),
                "wv": np.ascontiguousarray(wv),
                "wp": np.ascontiguousarray(wp),
                "ident": ident,
                "masktri": masktri,
            }
        )
    return in_maps


def kernel(x, W_attn, b_attn, W_proj, b_proj, _want_results=False, _spmd_kwargs=None):
    x = np.asarray(x, dtype=np.float32)
    W_attn = np.asarray(W_attn, dtype=np.float32)
    b_attn = np.asarray(b_attn, dtype=np.float32)
    W_proj = np.asarray(W_proj, dtype=np.float32)
    b_proj = np.asarray(b_proj, dtype=np.float32)

    if "nc" not in _CACHE:
        _CACHE["nc"] = _build()
    nc = _CACHE["nc"]

    in_maps = _prep_inputs(x, W_attn, b_attn, W_proj)
    kw = dict(_spmd_kwargs or {})
    res = run_bass_kernel_spmd(nc, in_maps, list(range(NCORES)), **kw)

    out = np.zeros((B, T, C), dtype=np.float32)
    for core in range(NCORES):
        out[core // 4] += res.results[core]["out"].astype(np.float32)
    out += b_proj[None, None, :]
    if _want_results:
        return out, res
    return out
